# revision 16
# baseline (speedup 1.0000x reference)
"""2-layer GAT on 8 Trainium2 NeuronCores — dma_gather edition.

Core c owns destination nodes [c*12500, (c+1)*12500); every edge lives on the
core that owns its destination. Per layer the per-node table (rows
[H | alpha_src | 1], grid cell order) is AllGathered compactly, re-pitched
locally to 256B rows, and the per-edge gather of table rows runs as chunked
InstDMAGatherAnt (<=8192 int16 indices per instruction, 4 SWDGE queues) from
4 table row-slices (so indices fit int16). Edges are bucketed per
(dst, slice) into 4 slot grids; each grid's weighted segment reduction is
regular DVE work, and the 4 partial cell tables are combined into one
canonical node table with dma_scatter_add. Softmax skips the max-subtraction
(logits bounded; identical to the stabilized form up to rounding).
"""
import sys

sys.path.insert(0, "/opt/trn_rl_repo")

import numpy as np

P = 128
N_NODES = 100000
N_CORES = 8
IN_DIM = 256
HID = 8
OUT = 16
NEG = 0.2
NSLICE = 4
NQ = 4                 # SWDGE queues
GCH = 64               # slot-cols per dma_gather instruction (8192 idx)
SCH = 48               # cell-cols per dma_scatter_add instruction (6144 idx)
ACH = 32               # cell-cols per alpha_d dma_gather instruction
TC = 128               # slot-cols per DVE chunk
RPB = 32               # rows/128 per repitch block
PITCH = 64             # fp32 per padded table row (256B)


ABLATE = set()     # timing ablation: subset of {"g", "a", "s"}


class _Meta:
    pass


def _dp_buckets(cnt, dmax):
    """cnt: [C, dmax+1] per-degree node counts. Returns deg->R map minimizing
    total slot columns sum_b ceil(max_c n_b/128)*R_b."""
    pred = cnt.cumsum(axis=1)
    INF = 1 << 60
    fdp = [0] + [INF] * dmax
    chx = [0] * (dmax + 1)
    for j in range(1, dmax + 1):
        for i in range(1, j + 1):
            n = pred[:, j] - pred[:, i - 1]
            v = fdp[i - 1] + int(np.ceil(n.max() / P)) * j
            if v < fdp[j]:
                fdp[j] = v
                chx[j] = i
    deg2R = np.zeros(dmax + 1, dtype=np.int64)
    j = dmax
    while j > 0:
        i = chx[j]
        deg2R[i:j + 1] = j
        j = i - 1
    return deg2R


def _wrap16(lin):
    """linear idx j -> tile[(j%16)+16g, j//16]; len(lin) % 16 == 0."""
    t = np.asarray(lin, np.int16).reshape(-1, 16).T
    return np.tile(t, (8, 1))


def _chunks_of(grid, target):
    """Split grid cell-cols into DVE chunks of <= target slot-cols, with
    per-chunk (R, cell-col range, slot-col base) intersections."""
    Rs, nrow = grid.Rs, grid.nrow
    colR, colsb = [], []
    acc = 0
    for R in Rs:
        for _ in range(nrow[R]):
            colR.append(R)
            colsb.append(acc)
            acc += R
    colsb.append(acc)
    ncol = len(colR)
    chunks = []
    i0 = 0
    while i0 < ncol:
        i1 = i0
        while i1 < ncol and colsb[i1 + 1] - colsb[i0] <= target:
            i1 += 1
        assert i1 > i0
        inters = []
        for R in Rs:
            ia = max(i0, grid.colbase[R])
            ib = min(i1, grid.colbase[R] + nrow[R])
            if ia < ib:
                inters.append((R, ia, ib, int(colsb[ia])))
        chunks.append((i0, i1, int(colsb[i0]), int(colsb[i1]), inters))
        i0 = i1
    return chunks


def _preprocess(E, X):
    N, C = N_NODES, N_CORES
    NLOC = N // C
    src = np.asarray(E[0], dtype=np.int64)
    dst = np.asarray(E[1], dtype=np.int64)

    # ---- main grid (one cell per node, R from total in-degree)
    deg = np.zeros((C, NLOC), dtype=np.int64)
    np.add.at(deg.reshape(-1), dst, 1)
    dmax = int(deg.max())
    cntd = np.zeros((C, dmax + 1), dtype=np.int64)
    for c in range(C):
        cntd[c] = np.bincount(deg[c][deg[c] > 0], minlength=dmax + 1)
    Rv = _dp_buckets(cntd, dmax)[deg]
    Rs_cells = sorted(set(int(r) for r in np.unique(Rv)), reverse=True)
    nrow = {}
    for R in Rs_cells:
        nrow[R] = int(np.ceil((Rv == R).sum(axis=1).max() / P))
    nt = sum(nrow.values()) + 1
    assert nt <= 127, nt
    NR = P * nt
    meta = _Meta()
    meta.N, meta.C, meta.NLOC, meta.NR, meta.nt = N, C, NLOC, NR, nt

    colbase = {}
    cb = 0
    for R in Rs_cells:
        colbase[R] = cb
        cb += nrow[R]
    cell2node = np.full((C, P, nt), -1, dtype=np.int64)
    tabrow_of = np.full(N, -1, dtype=np.int64)
    node_cell = np.full((C, NLOC), -1, dtype=np.int64)  # node -> p*nt+i
    for c in range(C):
        for R in Rs_cells:
            ls = np.nonzero(Rv[c] == R)[0]
            k = np.arange(len(ls))
            p = k % P
            i = colbase[R] + k // P
            cell2node[c, p, i] = ls
            tabrow_of[c * NLOC + ls] = c * NR + p * nt + i
            node_cell[c, ls] = p * nt + i
    meta.cell2node = cell2node
    meta.ones = (cell2node >= 0).astype(np.float32)

    XcT = np.zeros((C, IN_DIM, NR), dtype=np.float32)
    Xf = np.asarray(X, np.float32)
    for c in range(C):
        pp, ii = np.nonzero(cell2node[c] >= 0)
        ls = cell2node[c, pp, ii]
        XcT[c][:, ii * P + pp] = Xf[c * NLOC + ls].T
    meta.XcT = XcT

    # ---- slice grids (slice = src_row % NSLICE; packed idx = src_row // NSLICE
    # addresses the padded table [CNR//NSLICE, NSLICE*RW] at 256B-multiple
    # stride, so no repitch is needed)
    assert C * NR // NSLICE <= 32512
    src_row = tabrow_of[src]
    e_slice = src_row % NSLICE
    e_core = dst // NLOC
    dst_loc = dst % NLOC
    # per-slice dummy: any empty (all-zero) cell anywhere with residue s
    empt_c, empt_p, empt_i = np.nonzero(cell2node < 0)
    empt_row = empt_c * NR + empt_p * nt + empt_i
    dummy_of = {}
    for s in range(NSLICE):
        cand = empt_row[empt_row % NSLICE == s]
        assert len(cand), f"no empty cell with residue {s}"
        dummy_of[s] = int(cand[0]) // NSLICE
    grids = []
    for s in range(NSLICE):
        em = e_slice == s
        degS = np.zeros((C, NLOC), dtype=np.int64)
        np.add.at(degS.reshape(-1), dst[em], 1)
        dmx = int(degS.max())
        cnt = np.zeros((C, dmx + 1), dtype=np.int64)
        for c in range(C):
            cnt[c] = np.bincount(degS[c][degS[c] > 0], minlength=dmx + 1)
        RvS = _dp_buckets(cnt, dmx)[degS]

        g = _Meta()
        Rs = sorted((int(r) for r in np.unique(RvS) if r > 0), reverse=True)
        nrowS = {}
        for R in Rs:
            nrowS[R] = int(np.ceil((RvS == R).sum(axis=1).max() / P))
        g.Rs, g.nrow = Rs, nrowS
        g.colbase, g.slotbase = {}, {}
        cb = sb = 0
        for R in Rs:
            g.colbase[R] = cb
            g.slotbase[R] = sb
            cb += nrowS[R]
            sb += nrowS[R] * R
        g.ncol, g.T = cb, sb

        cellidx = np.full((C, NLOC), -1, dtype=np.int64)
        adidx = np.full((C, P, cb), -1, dtype=np.int64)
        for c in range(C):
            for R in Rs:
                ls = np.nonzero(RvS[c] == R)[0]
                k = np.arange(len(ls))
                cellidx[c, ls] = (g.colbase[R] + k // P) * P + (k % P)
                adidx[c, k % P, g.colbase[R] + k // P] = node_cell[c, ls]
        g.adidx = adidx

        idx = np.full((C, P, g.T), -1, dtype=np.int64)
        sr_all = (src_row // NSLICE)[em]
        dl_all = dst_loc[em]
        ec_all = e_core[em]
        for c in range(C):
            m = ec_all == c
            sr, dl = sr_all[m], dl_all[m]
            order = np.argsort(dl, kind="stable")
            sr, dl = sr[order], dl[order]
            grp = np.searchsorted(dl, np.arange(NLOC))
            pos = np.arange(len(dl)) - grp[dl]
            cell = cellidx[c, dl]
            R = RvS[c, dl]
            cp = cell % P
            ci = cell // P
            sb_arr = np.array([g.slotbase[int(r)] for r in R])
            cbs = np.array([g.colbase[int(r)] for r in R])
            t = sb_arr + (ci - cbs) * R + pos
            idx[c, cp, t] = sr
        g.idx = idx
        g.dummy_rel = dummy_of[s]
        g.chunks = _chunks_of(g, TC)
        # combine-gather: main cell (p,i) -> row in this grid's bucket table
        # (pitch ncol+1 cols per partition; col ncol is guaranteed zero)
        nc2 = g.ncol + 1
        cidx = np.full((C, P, nt), -1, dtype=np.int64)
        pgrid = np.arange(P)[:, None] * 0 + np.arange(P)[:, None]
        for c in range(C):
            cm = cellidx[c]            # node -> cell linear i*P+p, or -1
            tab = np.full((P, nt), -1, dtype=np.int64)
            pp2, ii2 = np.nonzero(cell2node[c] >= 0)
            nodes = cell2node[c, pp2, ii2]
            cl = cm[nodes]
            val = np.where(cl >= 0, (cl % P) * nc2 + cl // P, -1)
            tab[pp2, ii2] = val
            cidx[c] = np.where(tab >= 0, tab,
                               np.arange(P)[:, None] * nc2 + g.ncol)
        g.cidx = cidx
        grids.append(g)
    meta.grids = grids

    # ---- device idx streams, wrapped per instruction chunk
    JROW = P * nt          # junk row of canonical/ad tables
    gq, aq, sq = [], [], []
    gmeta, ameta, smeta = [], [], []
    for s, g in enumerate(grids):
        idx = np.where(g.idx >= 0, g.idx, g.dummy_rel)
        for (i0, i1, s0, s1, _) in g.chunks:
            for t0 in range(s0, s1, GCH):
                t1 = min(t0 + GCH, s1)
                blk = np.stack([
                    _wrap16(idx[c, :, t0:t1].T.reshape(-1)) for c in range(C)])
                gq.append(blk)
                gmeta.append((s, t0, t1))
        ad = np.where(g.adidx >= 0, g.adidx, JROW)
        for i0 in range(0, g.ncol, ACH):
            i1 = min(i0 + ACH, g.ncol)
            blk = np.stack([
                _wrap16(ad[c, :, i0:i1].T.reshape(-1)) for c in range(C)])
            aq.append(blk)
            ameta.append((s, i0, i1))
        for i0 in range(0, nt, GCH):
            i1 = min(i0 + GCH, nt)
            blk = np.stack([
                _wrap16(g.cidx[c, :, i0:i1].T.reshape(-1)) for c in range(C)])
            sq.append(blk)
            smeta.append((s, i0, i1))
    meta.gidx = np.concatenate(gq, axis=2)
    meta.aidx = np.concatenate(aq, axis=2)
    meta.sidx = np.concatenate(sq, axis=2)
    meta.gmeta, meta.ameta, meta.smeta = gmeta, ameta, smeta
    meta.JROW = JROW
    return meta


def _build(meta):
    import concourse.bass as bass
    import concourse.bacc as bacc
    import concourse.mybir as mybir
    import concourse.tile as tile
    from concourse.library_config import mlp

    F32 = mybir.dt.float32
    I16 = mybir.dt.int16
    AX = mybir.AxisListType
    OP = mybir.AluOpType
    AF = mybir.ActivationFunctionType

    C, NR, nt = meta.C, meta.NR, meta.nt
    D1, D2 = 16, 32                    # padded row [H | alpha_s | 1 | junk]
    KCH = IN_DIM // P
    CNR = C * NR
    CTROWS = P * nt + P                # canonical table rows (incl junk rows)
    grids = meta.grids

    nc = bacc.Bacc(num_swdge_queues=NQ)

    def raw_dma_gather(out_ap, in_ap, idxs_ap, num_idxs, elem_size,
                       elem_step, queue_num):
        gp = nc.gpsimd
        return gp.add_instruction(
            mybir.InstDMAGatherAnt(
                name=gp.bass.get_next_instruction_name(),
                ins=[*gp.lower_ap_dma(in_ap, for_custom_bir_dma=True),
                     gp.lower_ap(idxs_ap),
                     gp.lower_val_access(gp.to_reg(num_idxs))],
                outs=[gp.lower_ap(out_ap)],
                transpose=False, num_idxs=num_idxs, elem_size=elem_size,
                stride_bytes_256=(elem_step * 4) // 256,
                gen_mode=0, single_packet=False, queue_num=queue_num))

    XcT_d = nc.declare_dram_parameter("XcT", [IN_DIM, NR], F32, isOutput=False)
    ones_d = nc.declare_dram_parameter("ones", [P, nt], F32, isOutput=False)
    gidx_d = nc.declare_dram_parameter("gidx", list(meta.gidx.shape[1:]), I16,
                                       isOutput=False)
    aidx_d = nc.declare_dram_parameter("aidx", list(meta.aidx.shape[1:]), I16,
                                       isOutput=False)
    sidx_d = nc.declare_dram_parameter("sidx", list(meta.sidx.shape[1:]), I16,
                                       isOutput=False)
    W1_d = nc.declare_dram_parameter("W1", [IN_DIM, HID], F32, isOutput=False)
    a1s_d = nc.declare_dram_parameter("a1s", [1, HID], F32, isOutput=False)
    a1d_d = nc.declare_dram_parameter("a1d", [1, HID], F32, isOutput=False)
    W2T_d = nc.declare_dram_parameter("W2T", [1, OUT * HID], F32, isOutput=False)
    a2s_d = nc.declare_dram_parameter("a2s", [1, OUT], F32, isOutput=False)
    a2d_d = nc.declare_dram_parameter("a2d", [1, OUT], F32, isOutput=False)
    out_d = nc.declare_dram_parameter("out", [P, nt * OUT], F32, isOutput=True)

    cc1_d = nc.dram_tensor("cc1", [P, nt * D1], F32)
    tab1_d = nc.dram_tensor("tab1", [CNR // NSLICE, NSLICE * D1], F32,
                            addr_space="Shared")
    cc2_d = nc.dram_tensor("cc2", [P, nt * D2], F32)
    tab2_d = nc.dram_tensor("tab2", [CNR // NSLICE, NSLICE * D2], F32,
                            addr_space="Shared")
    adtab_d = nc.dram_tensor("adtab", [CTROWS, PITCH], F32)
    bt_d = [nc.dram_tensor(f"bt{s}", [P * (grids[s].ncol + 1), PITCH], F32)
            for s in range(NSLICE)]
    groups = [list(range(C))]

    def next_q():
        return 0   # rewritten post-schedule from the DMASW lane

    with tile.TileContext(nc) as tc:
        with (
            tc.tile_pool(name="persist", bufs=1) as pp,
            tc.tile_pool(name="xs", bufs=2) as xp,
            tc.tile_pool(name="gpl", bufs=4) as gpool,
            tc.tile_pool(name="ix", bufs=8) as ixp,
            tc.tile_pool(name="ew", bufs=4) as ewp,
            tc.tile_pool(name="us", bufs=2) as usp,
            tc.tile_pool(name="tmp", bufs=1) as tp,
            tc.tile_pool(name="ps", bufs=8, space="PSUM") as psp,
        ):
            nc.gpsimd.load_library(mlp)
            ones_t = pp.tile([P, nt], F32, tag="ones")
            nc.sync.dma_start(out=ones_t[:], in_=ones_d[:])
            w1_t = pp.tile([P, KCH * HID], F32, tag="w1")
            for k in range(KCH):
                nc.sync.dma_start(out=w1_t[:, k * HID:(k + 1) * HID],
                                  in_=W1_d[k * P:(k + 1) * P, :])
            a1s_t = pp.tile([P, HID], F32, tag="a1s")
            nc.sync.dma_start(out=a1s_t[:], in_=a1s_d[0:1, :].to_broadcast([P, HID]))
            a1d_t = pp.tile([P, HID], F32, tag="a1d")
            nc.sync.dma_start(out=a1d_t[:], in_=a1d_d[0:1, :].to_broadcast([P, HID]))
            w2t_t = pp.tile([P, OUT * HID], F32, tag="w2t")
            nc.sync.dma_start(out=w2t_t[:],
                              in_=W2T_d[0:1, :].to_broadcast([P, OUT * HID]))
            a2s_t = pp.tile([P, OUT], F32, tag="a2s")
            nc.sync.dma_start(out=a2s_t[:], in_=a2s_d[0:1, :].to_broadcast([P, OUT]))
            a2d_t = pp.tile([P, OUT], F32, tag="a2d")
            nc.sync.dma_start(out=a2d_t[:], in_=a2d_d[0:1, :].to_broadcast([P, OUT]))
            zt = pp.tile([P, PITCH], F32, tag="zero")
            nc.vector.memset(zt[:], 0.0)

            # ---- layer-1 node table (grid order): [H = X@W1 | alpha_s | 1]
            hg = pp.tile([P, nt * D1], F32, tag="hg")
            XB = 8
            for b0 in range(0, nt, XB):
                b1 = min(b0 + XB, nt)
                nb = b1 - b0
                xt = xp.tile([P, KCH * XB * P], F32, tag="xt")
                for k in range(KCH):
                    nc.sync.dma_start(out=xt[:, k * XB * P:k * XB * P + nb * P],
                                      in_=XcT_d[k * P:(k + 1) * P, b0 * P:b1 * P])
                for b in range(b0, b1):
                    ps = psp.tile([P, HID], F32, tag="hps")
                    for k in range(KCH):
                        nc.tensor.matmul(
                            out=ps[:],
                            lhsT=xt[:, k * XB * P + (b - b0) * P:
                                    k * XB * P + (b - b0 + 1) * P],
                            rhs=w1_t[:, k * HID:(k + 1) * HID],
                            start=(k == 0), stop=(k == KCH - 1))
                    nc.scalar.copy(out=hg[:, b * D1:b * D1 + HID], in_=ps[:])
            hv = hg[:].rearrange("p (n j) -> p n j", j=D1)[:, :, 0:HID]
            t_a = tp.tile([P, nt * HID], F32, tag="amul")
            tv = t_a[:].rearrange("p (n j) -> p n j", j=HID)
            nc.vector.tensor_tensor(out=tv, in0=hv,
                                    in1=a1s_t[:, None, :].to_broadcast([P, nt, HID]),
                                    op=OP.mult)
            nc.vector.tensor_reduce(
                out=hg[:].rearrange("p (n j) -> p n j", j=D1)[:, :, HID:HID + 1],
                in_=tv[:, :, None, :], axis=AX.X, op=OP.add)
            nc.vector.tensor_copy(
                out=hg[:].rearrange("p (n j) -> p n j", j=D1)[:, :, HID + 1:HID + 2],
                in_=ones_t[:, :, None])
            ad1_t = tp.tile([P, nt], F32, tag="ad1")
            t_b = tp.tile([P, nt * HID], F32, tag="amul2")
            tv2 = t_b[:].rearrange("p (n j) -> p n j", j=HID)
            nc.vector.tensor_tensor(out=tv2, in0=hv,
                                    in1=a1d_t[:, None, :].to_broadcast([P, nt, HID]),
                                    op=OP.mult)
            nc.vector.tensor_reduce(out=ad1_t[:, :, None], in_=tv2[:, :, None, :],
                                    axis=AX.X, op=OP.add)
            nc.sync.dma_start(out=cc1_d[:], in_=hg[:])
            if "c" not in ABLATE:
                nc.gpsimd.collective_compute(
                    "AllGather", OP.bypass, replica_groups=groups,
                    ins=[cc1_d[:]], outs=[tab1_d[:]])

            def layer(tab_d, D, ad_t, elem, gtag=""):
                """One GAT layer edge phase. ad_t: [P, nt] alpha_dst in grid
                order. Returns acc tile [P, nt, D+2] with z at col D+1."""
                Dc = D + 2
                # adtab: 256B-pitch, alpha_d at col 0 of row p*nt+i
                # (cols 1..15 stale garbage; the ad gather only reads col 0)
                adtv = adtab_d[0:P * nt, :].rearrange("(p n) j -> p n j", p=P)
                if "r" not in ABLATE:
                    nc.sync.dma_start(out=adtv[:, :, 0:1],
                                      in_=ad_t[:, :, None])
                    nc.sync.dma_start(
                        out=adtab_d[P * nt:CTROWS, 0:1].rearrange(
                            "(o p) j -> p (o j)", p=P),
                        in_=zt[:, 0:1])
                # ad gathers
                adg = [pp.tile([P, g.ncol], F32, tag=f"adg{s}", name=f"adg{s}")
                       for s, g in enumerate(grids)]
                acol = 0
                for ci, (s, i0, i1) in enumerate(meta.ameta):
                    w = i1 - i0
                    it = ixp.tile([P, 8 * w], I16, tag="ai")
                    nc.sync.dma_start(out=it[:],
                                      in_=aidx_d[:, acol:acol + 8 * w])
                    acol += 8 * w
                    gt = ewp.tile([P, w, 16], F32, tag="adgg")
                    if "a" not in ABLATE:
                        raw_dma_gather(gt[:], adtab_d[:, 0:16], it[:], P * w,
                                       16, PITCH, next_q())
                        nc.vector.tensor_copy(out=adg[s][:, i0:i1],
                                              in_=gt[:, :, 0])
                    else:
                        nc.vector.memset(adg[s][:, i0:i1], 0.0)
                # main gathers + edge math + partial scatter
                gchunks = list(meta.gmeta)
                gcol = [0]

                scol = 0
                sch_by_grid = {s: [] for s in range(NSLICE)}
                for (s, i0, i1) in meta.smeta:
                    sch_by_grid[s].append((i0, i1, scol))
                    scol += 8 * (i1 - i0)

                for s, g in enumerate(grids):
                    nc2 = g.ncol + 1
                    us = usp.tile([P, nc2, elem], F32, tag="us",
                                  bufs=1)
                    nc.vector.memset(us[:], 0.0)
                    for (i0, i1, s0, s1, inters) in g.chunks:
                        SC = s1 - s0
                        g_t = gpool.tile([P, TC, elem], F32, tag="g")
                        while (gchunks and gchunks[0][0] == s
                               and gchunks[0][1] < s1):
                            _, a, b = gchunks.pop(0)
                            w = b - a
                            off = gcol[0]
                            gcol[0] += 8 * w
                            it = ixp.tile([P, 8 * w], I16, tag="gi")
                            nc.sync.dma_start(out=it[:],
                                              in_=gidx_d[:, off:off + 8 * w])
                            if "g" not in ABLATE and ("g" + gtag) not in ABLATE:
                                raw_dma_gather(
                                    g_t[:, a - s0:a - s0 + w, :],
                                    tab_d[:, s * elem:(s + 1) * elem],
                                    it[:], P * w, elem, NSLICE * elem,
                                    next_q())
                        e_t = ewp.tile([P, SC], F32, tag="e")
                        w_t = ewp.tile([P, SC], F32, tag="w")
                        for (R, ia, ib, sa) in inters:
                            nn = ib - ia
                            o = sa - s0
                            ev = e_t[:, o:o + nn * R].rearrange(
                                "p (n r) -> p n r", r=R)
                            gv = g_t[:, o:o + nn * R, D].rearrange(
                                "p (n r) -> p n r", r=R)
                            adv2 = adg[s][:, ia:ib, None].to_broadcast(
                                [P, nn, R])
                            nc.vector.tensor_tensor(out=ev, in0=gv, in1=adv2,
                                                    op=OP.add)
                        nc.vector.tensor_scalar_mul(w_t[:, 0:SC], e_t[:, 0:SC],
                                                    NEG)
                        nc.vector.tensor_tensor(out=w_t[:, 0:SC],
                                                in0=w_t[:, 0:SC],
                                                in1=e_t[:, 0:SC], op=OP.max)
                        nc.scalar.activation(w_t[:, 0:SC], w_t[:, 0:SC], AF.Exp)
                        nc.vector.tensor_tensor(
                            out=g_t[:, 0:SC, 0:Dc],
                            in0=g_t[:, 0:SC, 0:Dc],
                            in1=w_t[:, 0:SC, None].to_broadcast([P, SC, Dc]),
                            op=OP.mult)
                        for (R, ia, ib, sa) in inters:
                            nn = ib - ia
                            o = sa - s0
                            uv = us[:, ia:ib, 0:Dc]
                            gv = g_t[:, o:o + nn * R, 0:Dc].rearrange(
                                "p (n r) j -> p n j r", r=R)
                            nc.vector.tensor_reduce(out=uv, in_=gv,
                                                    axis=AX.X, op=OP.add)
                    nc.sync.dma_start(
                        out=bt_d[s][:, 0:elem].rearrange(
                            "(p n) j -> p n j", p=P),
                        in_=us[:])
                # combine bucket tables into canonical acc via gathers
                acc = tp.tile([P, nt * Dc], F32, tag="acc")
                av = acc[:].rearrange("p (n j) -> p n j", j=Dc)
                for s in range(NSLICE):
                    for (i0, i1, off) in sch_by_grid[s]:
                        w = i1 - i0
                        it = ixp.tile([P, 8 * w], I16, tag="si")
                        nc.sync.dma_start(out=it[:],
                                          in_=sidx_d[:, off:off + 8 * w])
                        cgt = gpool.tile([P, TC, elem], F32, tag="g",
                                         name="cgt")
                        cg = cgt[:, 0:w, :]
                        if "s" not in ABLATE:
                            raw_dma_gather(cg, bt_d[s][:, 0:elem], it[:],
                                           P * w, elem, PITCH, next_q())
                        else:
                            nc.vector.memset(cg, 0.0)
                        if s == 0:
                            nc.vector.tensor_copy(out=av[:, i0:i1, :],
                                                  in_=cg[:, :, 0:Dc])
                        else:
                            nc.vector.tensor_tensor(out=av[:, i0:i1, :],
                                                    in0=av[:, i0:i1, :],
                                                    in1=cg[:, :, 0:Dc],
                                                    op=OP.add)
                return acc

            acc1 = layer(tab1_d, HID, ad1_t, 16, gtag="1")
            av1 = acc1[:].rearrange("p (n j) -> p n j", j=HID + 2)
            z_t = tp.tile([P, nt], F32, tag="z")
            nc.vector.tensor_scalar_add(z_t[:, :, None],
                                        av1[:, :, HID + 1:HID + 2], 1e-16)
            rec_t = tp.tile([P, nt], F32, tag="rec")
            nc.vector.reciprocal(rec_t[:], z_t[:])
            h2 = pp.tile([P, nt * HID], F32, tag="h2")
            h2v = h2[:].rearrange("p (n j) -> p n j", j=HID)
            nc.vector.tensor_tensor(
                out=h2v, in0=av1[:, :, 0:HID],
                in1=rec_t[:, :, None].to_broadcast([P, nt, HID]), op=OP.mult)
            tneg = tp.tile([P, nt * HID], F32, tag="telu")
            nc.vector.tensor_scalar_min(tneg[:], h2[:], 0.0)
            nc.scalar.activation(tneg[:], tneg[:], AF.Exp)
            nc.vector.tensor_scalar_max(h2[:], h2[:], 0.0)
            nc.vector.tensor_tensor(out=h2[:], in0=h2[:], in1=tneg[:], op=OP.add)
            nc.vector.tensor_scalar_add(h2[:], h2[:], -1.0)

            # ---- layer-2 node table
            hg2 = pp.tile([P, nt * D2], F32, tag="hg2")
            CB = 8
            for c0 in range(0, nt, CB):
                c1 = min(c0 + CB, nt)
                nn = c1 - c0
                tmw = tp.tile([P, CB * OUT * HID], F32, tag="tmw")
                tmv = tmw[:, :nn * OUT * HID].rearrange(
                    "p (n o j) -> p n o j", o=OUT, j=HID)
                nc.vector.tensor_tensor(
                    out=tmv,
                    in0=h2[:, c0 * HID:c1 * HID].rearrange(
                        "p (n j) -> p n j", j=HID)[:, :, None, :].to_broadcast(
                        [P, nn, OUT, HID]),
                    in1=w2t_t[:, None, :].to_broadcast(
                        [P, nn, OUT * HID]).rearrange("p n (o j) -> p n o j", o=OUT),
                    op=OP.mult)
                nc.vector.tensor_reduce(
                    out=hg2[:, c0 * D2:c1 * D2].rearrange(
                        "p (n j) -> p n j", j=D2)[:, :, 0:OUT],
                    in_=tmv, axis=AX.X, op=OP.add)
            hg2v = hg2[:].rearrange("p (n j) -> p n j", j=D2)
            ad2_t = tp.tile([P, nt], F32, tag="ad2")
            for (vec_t, dest) in ((a2s_t, hg2v[:, :, OUT:OUT + 1]),
                                  (a2d_t, ad2_t[:, :, None])):
                t_c = tp.tile([P, nt * OUT], F32, tag="amul3")
                tv3 = t_c[:].rearrange("p (n j) -> p n j", j=OUT)
                nc.vector.tensor_tensor(
                    out=tv3, in0=hg2v[:, :, 0:OUT],
                    in1=vec_t[:, None, :].to_broadcast([P, nt, OUT]), op=OP.mult)
                nc.vector.tensor_reduce(out=dest, in_=tv3[:, :, None, :],
                                        axis=AX.X, op=OP.add)
            nc.vector.tensor_copy(out=hg2v[:, :, OUT + 1:OUT + 2],
                                  in_=ones_t[:, :, None])
            nc.sync.dma_start(out=cc2_d[:], in_=hg2[:])
            if "c" not in ABLATE:
                nc.gpsimd.collective_compute(
                    "AllGather", OP.bypass, replica_groups=groups,
                    ins=[cc2_d[:]], outs=[tab2_d[:]])

            acc2 = layer(tab2_d, OUT, ad2_t, 32, gtag="2")
            av2 = acc2[:].rearrange("p (n j) -> p n j", j=OUT + 2)
            z2_t = tp.tile([P, nt], F32, tag="z2")
            nc.vector.tensor_scalar_add(z2_t[:, :, None],
                                        av2[:, :, OUT + 1:OUT + 2], 1e-16)
            rec2_t = tp.tile([P, nt], F32, tag="rec2")
            nc.vector.reciprocal(rec2_t[:], z2_t[:])
            o_t = pp.tile([P, nt * OUT], F32, tag="out")
            nc.vector.tensor_tensor(
                out=o_t[:].rearrange("p (n j) -> p n j", j=OUT),
                in0=av2[:, :, 0:OUT],
                in1=rec2_t[:, :, None].to_broadcast([P, nt, OUT]), op=OP.mult)
            nc.sync.dma_start(out=out_d[:], in_=o_t[:])
    # pin each SWDGE instruction's queue to its Tile DMASW lane (lane k ->
    # queue k%NQ) so every DMASW semaphore is driven by exactly one queue
    from concourse.tile_scheduler import PROC_NAMES
    lane_of = {i: n for i, n in enumerate(PROC_NAMES)}
    for f in nc.m.functions:
        for bb in f.blocks:
            for inst in bb.instructions:
                if isinstance(inst, (mybir.InstDMAGatherAnt,
                                     mybir.InstDMAScatterAddAnt)):
                    proc = getattr(inst, "bass_scheduled_proc", None)
                    name = lane_of.get(proc, "")
                    if name.startswith("DMASW"):
                        inst.queue_num = int(name[5:]) % NQ
    nc.finalize()
    return nc


def _in_maps(meta, W1, a1_src, a1_dst, W2, a2_src, a2_dst):
    maps = []
    for c in range(N_CORES):
        maps.append({
            "XcT": np.ascontiguousarray(meta.XcT[c]),
            "ones": np.ascontiguousarray(meta.ones[c]),
            "gidx": np.ascontiguousarray(meta.gidx[c]),
            "aidx": np.ascontiguousarray(meta.aidx[c]),
            "sidx": np.ascontiguousarray(meta.sidx[c]),
            "W1": np.asarray(W1, np.float32),
            "a1s": np.asarray(a1_src, np.float32).reshape(1, -1),
            "a1d": np.asarray(a1_dst, np.float32).reshape(1, -1),
            "W2T": np.ascontiguousarray(np.asarray(W2, np.float32).T).reshape(1, -1),
            "a2s": np.asarray(a2_src, np.float32).reshape(1, -1),
            "a2d": np.asarray(a2_dst, np.float32).reshape(1, -1),
        })
    return maps


def kernel(V, E, X, W1, a1_src, a1_dst, W2, a2_src, a2_dst):
    meta = _preprocess(E, X)
    nc = _build(meta)

    from concourse.bass_utils import run_bass_kernel_spmd

    in_maps = _in_maps(meta, W1, a1_src, a1_dst, W2, a2_src, a2_dst)
    res = run_bass_kernel_spmd(nc, in_maps, list(range(N_CORES)))

    out = np.zeros((N_NODES, OUT), dtype=np.float32)
    for c in range(N_CORES):
        g = res.results[c]["out"].reshape(P, meta.nt, OUT)
        pp, ii = np.nonzero(meta.cell2node[c] >= 0)
        ls = meta.cell2node[c, pp, ii]
        out[c * meta.NLOC + ls] = g[pp, ii]
    return out

